# revision 1
# baseline (speedup 1.0000x reference)
"""Bass/Trainium2 kernel for nn_Attention_1245540515949.

Reference computation (B=32, T=4096, H=512), fp32 inputs:
    cat    = concat([broadcast(hidden), enc], -1)          # [B,T,2H]
    energy = softmax(cat @ W_attn.T + b_attn, axis=0)      # batch-dim softmax!
    scores = relu(einsum('h,bth->bt', v, energy))[:, None] # [B,1,T]

Strategy: shard T across the 8 cores (the batch softmax stays core-local).
Device math in bf16 (fp32 accumulation in PSUM / reductions), per core:

  E^T[h_out, (t,b)] = W2T.T @ encT          PE, bf16, k-chunked, N=512
       + A'[b, h_out]                        K=32 "indicator" matmuls,
                                             row-packed via tile_position
    where A' = hidden @ W1.T + b_attn        tiny on-device matmul (col-packed
                                             4x down partitions -> A'rep)
  X   = exp(E^T + A')                        ScalarE, PSUM->SBUF bf16
  den[t,h] = sum_b X                         DVE segmented reduce (b inner)
  U[h,t]   = v[h] / den[t,h]                 DVE reciprocal + per-part scalar
  scores[t,b] = sum_h U[h,t] * X[h,(t,b)]    per-t K=128,M=1,N=32 matmuls,
                                             col-packed 4x -> psum partitions
  relu                                       ScalarE, one [128,512] op / 4 nt

encT is laid out host-side as [H, Tc*B] bf16 so each DMA is 128 partitions
x 1KB contiguous and no on-device transpose is needed; HBM traffic is
16.5 MiB/core.
"""

import numpy as np

B, T, H = 32, 4096, 512
NCORES = 8
TC = T // NCORES          # 512 t-values per core
P = 128                   # partitions
NT = (TC * B) // 512      # 32 n-tiles of 512 (t,b) pairs
TPT = 512 // B            # 16 t-values per n-tile
KC = H // P               # 4 k-chunks
MC = H // P               # 4 m-chunks (h_out)
NTG = 4                   # n-tiles per scores psum bank
SCW = NTG * P             # scores columns per bank (512)

_CACHE = {}


def _build_nc():
    import concourse.mybir as mybir
    from concourse.bacc import Bacc
    from concourse.tile import TileContext

    f32 = mybir.dt.float32
    bf16 = mybir.dt.bfloat16
    AF = mybir.ActivationFunctionType
    AX = mybir.AxisListType

    nc = Bacc()

    encT = nc.declare_dram_parameter("enct", [H, TC * B], bf16, isOutput=False)
    w2t = nc.declare_dram_parameter("w2t", [H, H], bf16, isOutput=False)
    w1t = nc.declare_dram_parameter("w1t", [H, H], bf16, isOutput=False)
    hidT = nc.declare_dram_parameter("hidt", [H, B], bf16, isOutput=False)
    brow = nc.declare_dram_parameter("brow", [1, H], bf16, isOutput=False)
    ones = nc.declare_dram_parameter("ones", [1, B], bf16, isOutput=False)
    ind = nc.declare_dram_parameter("ind", [P, 512], bf16, isOutput=False)
    vcol = nc.declare_dram_parameter("vcol", [P, MC], f32, isOutput=False)
    out = nc.declare_dram_parameter("scores", [16, NT // NTG * SCW], f32,
                                    isOutput=True)

    with TileContext(nc) as tc:
        with (
            tc.tile_pool(name="consts", bufs=1) as consts,
            tc.tile_pool(name="enc", bufs=NT * KC) as encp,
            tc.tile_pool(name="xs", bufs=12) as xp,
            tc.tile_pool(name="dens", bufs=24) as dp,
            tc.tile_pool(name="mainps", bufs=4, space="PSUM") as psp,
            tc.tile_pool(name="scps", bufs=3, space="PSUM") as scp,
            tc.tile_pool(name="apps", bufs=1, space="PSUM") as app,
        ):
            # ---- constants into SBUF ----
            w2t_sb, w1t_sb, hid_sb = [], [], []
            for kc in range(KC):
                t_ = consts.tile([P, H], bf16, name=f"w2t{kc}")
                nc.sync.dma_start(out=t_, in_=w2t[kc * P:(kc + 1) * P, :])
                w2t_sb.append(t_)
                t_ = consts.tile([P, H], bf16, name=f"w1t{kc}")
                nc.sync.dma_start(out=t_, in_=w1t[kc * P:(kc + 1) * P, :])
                w1t_sb.append(t_)
                t_ = consts.tile([P, B], bf16, name=f"hidt{kc}")
                nc.sync.dma_start(out=t_, in_=hidT[kc * P:(kc + 1) * P, :])
                hid_sb.append(t_)
            brow_sb = consts.tile([1, H], bf16, name="brow")
            nc.sync.dma_start(out=brow_sb, in_=brow[:, :])
            ones_sb = consts.tile([1, B], bf16, name="ones")
            nc.sync.dma_start(out=ones_sb, in_=ones[:, :])
            ind_sb = consts.tile([P, 512], bf16, name="ind")
            nc.sync.dma_start(out=ind_sb, in_=ind[:, :])
            vcol_sb = consts.tile([P, MC], f32, name="vcol")
            nc.sync.dma_start(out=vcol_sb, in_=vcol[:, :])
            scores_sb = consts.tile([P, NT // NTG * SCW], f32,
                                    name="scores_sb")
            # prewarm the exp table set so the ~2.7us ACT_TABLE_LOAD overlaps
            # the enc prefetch instead of stalling the first tile
            warm = consts.tile([1, 1], f32, name="actwarm")
            nc.scalar.activation(out=warm, in_=vcol_sb[0:1, 0:1], func=AF.Exp)

            # ---- A'rep: A' = hidden @ W1.T + b_attn, replicated to the 4
            #      32-row groups (col-packed matmuls) ----
            # single start/stop per PSUM bank: start clears the whole bank's
            # has_written bits; per-element semantics handle the rest
            # per-col-group accumulation groups: start/stop scope to each
            # group's own partition rows (per-partition has_written regions)
            ap_ps = app.tile([P, H], f32, name="ap_ps")
            for g in range(4):
                for kc in range(KC):
                    nc.tensor.matmul(
                        out=ap_ps[32 * g:32 * (g + 1), :],
                        lhsT=hid_sb[kc], rhs=w1t_sb[kc],
                        start=(kc == 0), stop=False,
                        tile_position=(0, 32 * g),
                    )
                nc.tensor.matmul(
                    out=ap_ps[32 * g:32 * (g + 1), :],
                    lhsT=ones_sb, rhs=brow_sb, start=False, stop=True,
                    tile_position=(0, 32 * g),
                )
            aprep_sb = consts.tile([P, H], bf16, name="aprep")
            nc.vector.tensor_copy(out=aprep_sb, in_=ap_ps)

            # ---- main loop ----
            for nt in range(NT):
                e_sb = []
                for kc in range(KC):
                    t_ = encp.tile([P, 512], bf16, tag="enc")
                    nc.sync.dma_start(
                        out=t_,
                        in_=encT[kc * P:(kc + 1) * P, nt * 512:(nt + 1) * 512],
                    )
                    e_sb.append(t_)

                sc_ps = scp.tile([P, P], f32, tag="scps")

                ps_tiles = []
                for mc in range(MC):
                    ps = psp.tile([P, 512], f32, tag="mainps")
                    for kc in range(KC):
                        nc.tensor.matmul(
                            out=ps,
                            lhsT=w2t_sb[kc][:, mc * P:(mc + 1) * P],
                            rhs=e_sb[kc],
                            start=(kc == 0), stop=False,
                        )
                    ps_tiles.append(ps)
                # A' add: 4 row-packed K=32 matmuls (concurrent row groups)
                for mc in range(MC):
                    nc.tensor.matmul(
                        out=ps_tiles[mc],
                        lhsT=aprep_sb[32 * mc:32 * (mc + 1),
                                      mc * P:(mc + 1) * P],
                        rhs=ind_sb[32 * mc:32 * (mc + 1), :],
                        start=False, stop=True,
                        tile_position=(32 * mc, 0),
                    )

                x_tiles, u_tiles = [], []
                for mc in range(MC):
                    x = xp.tile([P, 512], bf16, tag="x")
                    nc.scalar.activation(out=x, in_=ps_tiles[mc], func=AF.Exp)
                    x3 = x.rearrange("p (t b) -> p t b", b=B)

                    den = dp.tile([P, TPT], f32, tag="den")
                    nc.vector.reduce_sum(out=den, in_=x3, axis=AX.X)
                    rden = dp.tile([P, TPT], f32, tag="rden")
                    nc.vector.reciprocal(out=rden, in_=den)
                    u = dp.tile([P, TPT], bf16, tag="u")
                    nc.vector.tensor_scalar_mul(
                        out=u, in0=rden, scalar1=vcol_sb[:, mc:mc + 1],
                    )
                    x_tiles.append(x)
                    u_tiles.append(u)

                # scores: per r-block of 4 t's, K=128(mc-chunk),M=4,N=128
                # matmuls; out[j, 32j:32j+32] is the valid diagonal
                # (t = nt*16 + 4r + j); off-diagonal strips are scratch the
                # host ignores. Col-packed onto psum partition groups
                # {0,32,64,96}; r outer so each group start->stop completes
                # before the next group's start clears has_written bits.
                for r in range(4):
                    for mc in range(MC):
                        nc.tensor.matmul(
                            out=sc_ps[32 * r:32 * r + 4, :],
                            lhsT=u_tiles[mc][:, 4 * r:4 * (r + 1)],
                            rhs=x_tiles[mc][:, P * r:P * (r + 1)],
                            start=(mc == 0), stop=(mc == 3),
                            tile_position=(0, 32 * r),
                        )

                # relu only the 4-row groups the t-matmuls wrote (rest of the
                # bank is never initialized); ACT cost is free-dim-based so
                # four [4,128] ops cost the same per-op as full-tile ones
                for r in range(4):
                    nc.scalar.activation(
                        out=scores_sb[32 * r:32 * r + 4, nt * P:(nt + 1) * P],
                        in_=sc_ps[32 * r:32 * r + 4, :], func=AF.Relu,
                    )

            # SWDGE for the out-DMAs: they need waits on both ACT (relu) and
            # the DMA lane, and the HWDGE direct2d pseudo only takes one.
            # Only the 4-row groups the relus wrote are shipped (256 KB, not
            # the full 2 MiB tile) to shorten the kernel tail.
            for r in range(4):
                nc.gpsimd.dma_start(
                    out=out[4 * r:4 * (r + 1), :],
                    in_=scores_sb[32 * r:32 * r + 4, :],
                )

    # bacc passes: reg alloc + move_matmul_waits_to_ldweights etc. — without
    # this, walrus codegen rejects instructions carrying >1 sync wait
    nc.compile()
    return nc


def _prep_inputs(hidden, encoder_outputs, W_attn, b_attn, v):
    """Host-side shard + layout prep. Returns in_maps for the 8 cores."""
    import ml_dtypes
    bf16 = ml_dtypes.bfloat16

    hidden = np.asarray(hidden, dtype=np.float32)
    enc = np.asarray(encoder_outputs, dtype=np.float32)
    W = np.asarray(W_attn, dtype=np.float32)
    b = np.asarray(b_attn, dtype=np.float32)
    v = np.asarray(v, dtype=np.float32)

    w1t = np.ascontiguousarray(W[:, :H].T).astype(bf16)   # [k1, h_out]
    w2t = np.ascontiguousarray(W[:, H:].T).astype(bf16)   # [h_in, h_out]
    hidT = np.ascontiguousarray(hidden.T).astype(bf16)    # [H, B]
    brow = b[None, :].astype(bf16)                        # [1, H]
    ones = np.ones((1, B), bf16)
    ind = np.tile(np.eye(B, dtype=np.float32), (4, 512 // B)).astype(bf16)
    vcol = np.ascontiguousarray(v.reshape(MC, P).T)       # [P, MC] f32

    in_maps = []
    for c in range(NCORES):
        shard = enc[c * TC:(c + 1) * TC]                  # [TC, B, H]
        encT = np.ascontiguousarray(
            shard.reshape(TC * B, H).T).astype(bf16)      # [H, TC*B]
        in_maps.append({
            "enct": encT, "w2t": w2t, "w1t": w1t, "hidt": hidT,
            "brow": brow, "ones": ones, "ind": ind, "vcol": vcol,
        })
    return in_maps


def _assemble(results):
    """results: per-core dicts with 'scores' [128, 4096].

    score(t_loc = nt*16 + 4r + j, b) lives at
    scores[32r + j, (nt//4)*512 + (nt%4)*128 + 32j + b]; everything else
    in the tensor is matmul scratch.
    """
    out = np.empty((B, 1, T), np.float32)
    for c in range(NCORES):
        # dram rows 4r+j hold sbuf rows 32r+j: [r, j_row, grp, ntl, j_col, b]
        s = results[c]["scores"].reshape(4, 4, NT // NTG, NTG, 4, B)
        # valid slots are the j_row == j_col diagonal
        s = np.stack([s[:, j, :, :, j, :] for j in range(4)], axis=1)
        # s: [r, j, grp, ntl, b];  t_loc = (grp*4 + ntl)*16 + 4r + j
        s = s.transpose(2, 3, 0, 1, 4)                  # [grp, ntl, r, j, b]
        out[:, 0, c * TC:(c + 1) * TC] = s.reshape(TC, B).T
    return out


def run(in_maps, trace=False, **kw):
    from concourse.bass_utils import run_bass_kernel_spmd

    if "nc" not in _CACHE:
        _CACHE["nc"] = _build_nc()
    nc = _CACHE["nc"]
    return run_bass_kernel_spmd(
        nc, in_maps, list(range(NCORES)), trace=trace, **kw
    )


def kernel(hidden, encoder_outputs, W_attn, b_attn, v):
    in_maps = _prep_inputs(hidden, encoder_outputs, W_attn, b_attn, v)
    br = run(in_maps)
    return _assemble(br.results)

